# revision 16
# baseline (speedup 1.0000x reference)
"""MinibatchDiscrimination kernel for 8 Trainium2 NeuronCores.

reference:
    m = einsum('bi,iok->bok', x, T)          # B=128, IN=1024, OUT=512, K=16
    norm[i,j,o] = sum_k |m[j,o,k] - m[i,o,k]|
    o_b = sum_i exp(-norm) - 1               # [B, OUT]
    out = concat([x, o_b], axis=1)           # [128, 1536]

Sharding: each core owns OUT/8 = 64 output features (zero communication).

Strictly-upper pairs (i<j) are split by i into two pipelines so all three
engines (PE / ACT / DVE) carry comparable load:

Pipeline A (i in blocks {0,1,6,7} of 16 -> 4064 pairs), per f-tile t:
  pair-diff on PE ([f, pairs] layout) -> |.| on ACT -> k-reduce on PE
  (packed selector matmuls, 4-strip interleave) -> exp on ACT over the
  [i-packed, j] grid -> col sums on PE (s2 selector) + row sums on DVE.
  PSUM pnorm grids are memset once and reuse the same written-mask, so
  stale cells stay exactly 0 -> exp gives exactly 1.0 junk, removed
  host-side via known counts.

Pipeline B (i in 32..95 -> 4064 pairs + 32 zero-pad), per 128-pair chunk:
  pair-diff transposed on PE (lhsT=psel chunk -> [pairs, f] f32 PSUM)
  -> fused |.|+k-reduce on DVE (tensor_reduce apply_absolute_value)
  -> exp on ACT ([pairs, o]) -> j-sum on PE (|psel|^T selector) which
  yields exact per-j sums with no junk (pad columns have zero weights).

Host combines: o_b[j, o] = A-colsums + A-rowsums (reindexed) + B-jsums.
"""

import numpy as np
import ml_dtypes

import concourse.bass as bass
import concourse.tile as tile
from concourse import mybir
from concourse.bass_utils import run_bass_kernel_spmd

BF16 = mybir.dt.bfloat16
F32 = mybir.dt.float32
A = mybir.AluOpType
AF = mybir.ActivationFunctionType
AX = mybir.AxisListType

B = 128
IN = 1024
OUT = 512
K = 16
NCORES = 8
OC = OUT // NCORES       # 64
F = OC * K               # 1024
NT = F // 128            # 8 f-tiles
NCI = IN // 128          # 8 contraction chunks

A_IGS = (0, 1, 6, 7)                    # 16-i blocks handled by pipeline A
B_IGS = tuple(g for g in range(8) if g not in A_IGS)
A_SET = tuple(i for g in A_IGS for i in range(16 * g, 16 * g + 16))
B_SET = tuple(i for g in B_IGS for i in range(16 * g, 16 * g + 16))
NPA = sum(127 - i for i in A_SET)       # 4064
NPB_RAW = sum(127 - i for i in B_SET)   # 4064
NBCH = (NPB_RAW + 127) // 128           # 32 B chunks
NPB = 128 * NBCH                        # padded with zero-selector columns
NPTOT = NPA + NPB                       # 8160
SUPW = 508
NSUP = NPA // SUPW                      # 8 supers per tile (exact: 4064=8*508)
assert NSUP * SUPW == NPA

# pair base offsets within the A region (i ascending over A_SET)
_BASE_A = {}
_off = 0
for _i in sorted(A_SET):
    _BASE_A[_i] = _off
    _off += 127 - _i


def _split_excess_waits(nc, max_waits=1):
    """This walrus build rejects instructions carrying more than one sem
    wait; hoist extras onto preceding NoOps on the same engine."""
    for fn in nc.m.functions:
        for blk in fn.blocks:
            new_insts = []
            for inst in blk.instructions:
                si = inst.sync_info
                if si and si.on_wait and len(si.on_wait) > max_waits:
                    waits = list(si.on_wait)
                    extra, keep = waits[:-max_waits], waits[-max_waits:]
                    k = 0
                    while extra:
                        chunk, extra = extra[:max_waits], extra[max_waits:]
                        nop = mybir.InstNoOp(
                            name=f"{inst.name}-ws{k}", engine=inst.engine,
                            ins=[], outs=[],
                            sync_info=mybir.SyncInfo(on_wait=chunk, on_update=[]))
                        nc.register_instruction(nop)
                        new_insts.append(nop)
                        k += 1
                    inst.sync_info = mybir.SyncInfo(
                        on_wait=keep, on_update=list(si.on_update))
                new_insts.append(inst)
            blk.instructions[:] = new_insts


def _build_program():
    nc = bass.Bass()
    xt_d = nc.dram_tensor("xt", [128, NCI, B], BF16, kind="ExternalInput")
    tc_d = nc.dram_tensor("tc", [128, 2, NCI, 512], BF16, kind="ExternalInput")
    psel_d = nc.dram_tensor("psel", [B, NPTOT], BF16, kind="ExternalInput")
    pat_d = nc.dram_tensor("pat", [128, NBCH, 128], BF16, kind="ExternalInput")
    s32_d = nc.dram_tensor("s32", [128, 4, 32], BF16, kind="ExternalInput")
    s2_d = nc.dram_tensor("s2", [128, NT, OC], BF16, kind="ExternalInput")
    po_d = nc.dram_tensor("po", [128, 192], F32, kind="ExternalOutput")
    rs_d = nc.dram_tensor("rs", [128, 32], F32, kind="ExternalOutput")

    with tile.TileContext(nc) as tc:
        with (
            tc.tile_pool(name="cst", bufs=1) as cst,
            tc.tile_pool(name="work", bufs=3) as work,
            tc.tile_pool(name="ework", bufs=4) as ework,
            tc.tile_pool(name="normb", bufs=2) as normb,
            tc.tile_pool(name="ebuf", bufs=2) as ebuf,
            tc.tile_pool(name="pdiff", bufs=2, space="PSUM") as pdiff,
            tc.tile_pool(name="pnorm", bufs=2, space="PSUM") as pnorm,
            tc.tile_pool(name="pbd", bufs=2, space="PSUM") as pbd,
            tc.tile_pool(name="pobA", bufs=1, space="PSUM") as pobA,
            tc.tile_pool(name="pobB", bufs=1, space="PSUM") as pobB,
        ):
            # ---- inputs: few large DMAs, dependency-ordered, two queues ----
            xt_sb = cst.tile([128, NCI, B], BF16, tag="xt")
            tc_sb = cst.tile([128, 2, NCI, 512], BF16, tag="tc")
            psel_sb = cst.tile([128, NPTOT], BF16, tag="psel")
            pat_sb = cst.tile([128, NBCH, 128], BF16, tag="pat")
            s32_sb = cst.tile([128, 4, 32], BF16, tag="s32")
            s2_sb = cst.tile([128, NT, OC], BF16, tag="s2")
            nc.sync.dma_start(xt_sb[:], xt_d[:])
            nc.sync.dma_start(tc_sb[:, 0, 0:4], tc_d[:, 0, 0:4])
            nc.sync.dma_start(tc_sb[:, 0, 4:8], tc_d[:, 0, 4:8])
            nc.sync.dma_start(tc_sb[:, 1], tc_d[:, 1])
            nc.scalar.dma_start(s32_sb[:], s32_d[:])
            nc.scalar.dma_start(psel_sb[:, 0:1016], psel_d[:, 0:1016])
            nc.scalar.dma_start(psel_sb[:, 1016:NPA], psel_d[:, 1016:NPA])
            nc.scalar.dma_start(s2_sb[:], s2_d[:])
            nc.scalar.dma_start(psel_sb[:, NPA:NPTOT], psel_d[:, NPA:NPTOT])
            nc.scalar.dma_start(pat_sb[:], pat_d[:])

            # ---- PE warm-up: the HAM clock gate keeps the PE at 1.2 GHz
            # until ~3.4us of sustained activity.  s32 lands first (tiny,
            # first on the scalar DMA queue), so dummy matmuls on it keep
            # the PE busy through the tc/xt DMA wait; GEMM then runs at
            # the full 2.4 GHz.  Output goes to a pnorm bank that is
            # memset before its first real use (buffer parity for the
            # stale-mask trick is preserved: t and t+2 still share a bank).
            pn_warm = pnorm.tile([128, 512], F32, tag="pn")
            s32_flat = s32_sb[:].rearrange("p a c -> p (a c)")
            for wu in range(48):
                nc.tensor.matmul(pn_warm[0:32, 0:128], s32_sb[:, wu % 4, :],
                                 s32_flat, start=True, stop=True,
                                 skip_group_check=True)

            # ---- GEMM: m[b, f] = x @ T_c (PSUM shared with pipeline B) ----
            m_bf = cst.tile([128, F], BF16, tag="mbf")
            for half in range(2):
                ps = pbd.tile([128, 512], F32, tag="bpd")
                for ci in range(NCI):
                    nc.tensor.matmul(
                        ps[:], xt_sb[:, ci, :], tc_sb[:, half, ci, :],
                        start=(ci == 0), stop=(ci == NCI - 1))
                nc.scalar.activation(m_bf[:, 512 * half:512 * (half + 1)],
                                     ps[:], AF.Copy, scale=1.0)

            po_a = pobA.tile([64, 512], F32, tag="poa")
            po_b = pobB.tile([128, 512], F32, tag="pob")
            rs_all = cst.tile([128, 4, 8], F32, tag="rs")

            # ---------- pipeline A helpers ----------
            def make_pd_steps(t):
                absd = work.tile([128, NPA], BF16, tag="absd")

                def step(s):
                    pd = pdiff.tile([128, 512], F32, tag="pd")
                    lo = s * SUPW
                    nc.tensor.matmul(pd[:, 0:SUPW],
                                     m_bf[:, 128 * t:128 * (t + 1)],
                                     psel_sb[:, lo:lo + SUPW],
                                     start=True, stop=True)
                    nc.scalar.activation(absd[:, lo:lo + SUPW],
                                         pd[:, 0:SUPW], AF.Abs)

                return absd, [lambda s=s: step(s) for s in range(NSUP)]

            # ---------- pipeline B helpers ----------
            bstate = {}

            def bstep(c):
                if c >= NBCH:
                    return
                if c % 4 == 0:
                    bstate["nb"] = normb.tile([128, 4, OC], F32, tag="nb",
                                              name="nb")
                nb = bstate["nb"]
                for h in range(2):
                    pd = pbd.tile([128, 512], F32, tag="bpd")
                    nc.tensor.matmul(pd[:], psel_sb[:, NPA + 128 * c:
                                                     NPA + 128 * (c + 1)],
                                     m_bf[:, 512 * h:512 * (h + 1)],
                                     start=True, stop=True)
                    nc.vector.tensor_reduce(
                        nb[:, c % 4, 32 * h:32 * (h + 1)],
                        pd[:].rearrange("p (o k) -> p o k", k=K),
                        op=A.add, axis=AX.X, apply_absolute_value=True)
                if c % 4 == 3:
                    eb = ebuf.tile([128, 4, OC], BF16, tag="eb", name="eb")
                    nc.scalar.activation(eb[:], nb[:], AF.Exp, scale=-1.0)
                    for k2 in range(4):
                        cc = (c // 4) * 4 + k2
                        nc.tensor.matmul(
                            po_b[:, 0:OC], pat_sb[:, cc, :], eb[:, k2, :],
                            start=(cc == 0), stop=(cc == NBCH - 1),
                            skip_group_check=True)

            # ---------- pipeline A k-reduce + exp + sums ----------
            def kred(t, absd, weave):
                weave = list(weave)
                stride = max(1, 64 // (len(weave) + 1)) if weave else 0
                mm_count = 0

                def tick():
                    nonlocal mm_count
                    mm_count += 1
                    if weave and stride and mm_count % stride == 0:
                        weave.pop(0)()

                pn = pnorm.tile([128, 512], F32, tag="pn")
                if t < 2:
                    # only the first use of each pnorm bank needs zeroing:
                    # later rounds rewrite the identical cell mask, so
                    # never-written cells keep this exact 0.0 forever
                    nc.vector.memset(pn[:], 0.0)
                first = True
                for gl in range(4):
                    ig = A_IGS[gl]
                    for idx in range(16):
                        q, a = idx % 4, idx // 4
                        i = 16 * ig + 4 * a + q
                        w = 127 - i
                        if w <= 0:
                            continue
                        bs = _BASE_A[i]
                        last = (gl == 3 and idx == 15)
                        nc.tensor.matmul(
                            pn[32 * q:32 * q + 32,
                               128 * gl + i + 1:128 * (gl + 1)],
                            s32_sb[:, a, :], absd[:, bs:bs + w],
                            start=first, stop=last,
                            tile_position=(0, 32 * q), skip_group_check=True)
                        first = False
                        tick()
                e = ework.tile([128, 512], BF16, tag="e")
                nc.scalar.activation(e[:], pn[:], AF.Exp, scale=-1.0)
                nc.vector.tensor_reduce(
                    rs_all[:, :, t],
                    e[:].rearrange("p (g j) -> p g j", g=4),
                    op=A.add, axis=AX.X)
                for gl in range(4):
                    nc.tensor.matmul(po_a[:, 0:128], s2_sb[:, t, :],
                                     e[:, 128 * gl:128 * (gl + 1)],
                                     start=(t == 0 and gl == 0),
                                     stop=(t == NT - 1 and gl == 3),
                                     skip_group_check=True)
                for stp in weave:
                    stp()

            # ---------- main schedule ----------
            absd0, steps0 = make_pd_steps(0)
            for s in steps0:
                s()
            cur_absd = absd0
            for t in range(NT):
                if t + 1 < NT:
                    nxt_absd, nxt_steps = make_pd_steps(t + 1)
                else:
                    nxt_absd, nxt_steps = None, []
                bsteps = [lambda c=c: bstep(c) for c in range(4 * t, 4 * t + 4)]
                # interleave pd (ACT abs) and B (DVE reduce) work so both
                # side engines stay fed during this tile's k-reduce
                weave = []
                for x_ in range(max(len(nxt_steps), len(bsteps))):
                    if x_ < len(nxt_steps):
                        weave.append(nxt_steps[x_])
                    if x_ < len(bsteps):
                        weave.append(bsteps[x_])
                kred(t, cur_absd, weave)
                cur_absd = nxt_absd

            po_sb = cst.tile([128, 192], F32, tag="posb")
            nc.vector.memset(po_sb[:], 0.0)
            nc.vector.tensor_copy(po_sb[0:64, 0:128], po_a[:, 0:128])
            nc.vector.tensor_copy(po_sb[:, 128:192], po_b[:, 0:OC])
            nc.sync.dma_start(po_d[:], po_sb[:])
            nc.sync.dma_start(rs_d[:], rs_all[:])

    _split_excess_waits(nc)
    return nc


def _host_consts():
    bf = ml_dtypes.bfloat16
    order = [(i, j) for i in sorted(A_SET) for j in range(i + 1, 128)]
    order += [(i, j) for i in sorted(B_SET) for j in range(i + 1, 128)]
    psel = np.zeros((B, NPTOT), np.float32)
    for col, (i, j) in enumerate(order):
        psel[i, col] = 1.0
        psel[j, col] = -1.0
    pat = np.zeros((128, NBCH, 128), np.float32)
    for c in range(NBCH):
        for r in range(128):
            gcol = NPA + 128 * c + r
            if gcol - NPA < NPB_RAW:
                i, j = order[gcol]
                pat[r, c, i] = 1.0
                pat[r, c, j] = 1.0
    s32 = np.zeros((128, 4, 32), np.float32)
    for a in range(4):
        for osub in range(8):
            s32[16 * osub:16 * (osub + 1), a, 8 * a + osub] = 1.0
    s2 = np.zeros((128, NT, OC), np.float32)
    for t in range(NT):
        for p in range(128):
            s2[p, t, 8 * t + (p % 8)] = 1.0
    return (psel.astype(bf), pat.astype(bf), s32.astype(bf), s2.astype(bf))


_CACHE = {}


def _get_cached():
    if "nc" not in _CACHE:
        _CACHE["nc"] = _build_program()
        _CACHE["consts"] = _host_consts()
        # A-side junk: unwritten (j <= i) cells read exp(0)=1.0
        jj = np.arange(B)
        a_arr = np.array(sorted(A_SET))
        _CACHE["cntA"] = (a_arr[None, :] >= jj[:, None]).sum(1).astype(np.float32)
        # rowsum reindex: rs[p, gl, t] belongs to i = 16*A_IGS[gl] + 4a + q
        # with p = 32q + 8a + osub, o = 8t + osub
        p_idx = np.arange(128)
        q, rem = p_idx // 32, p_idx % 32
        a_, osub = rem // 8, rem % 8
        cols = np.arange(32)
        gl, t_ = cols // 8, cols % 8
        igs = np.array(A_IGS)
        i_map = 16 * igs[gl][None, :] + 4 * a_[:, None] + q[:, None]
        o_map = 8 * t_[None, :] + osub[:, None]
        _CACHE["i_map"] = i_map
        _CACHE["o_map"] = o_map
    return _CACHE


def kernel(x: np.ndarray, T: np.ndarray, _trace=False, _tmpdir=None) -> np.ndarray:
    x = np.asarray(x, dtype=np.float32)
    T = np.asarray(T, dtype=np.float32)
    c = _get_cached()
    nc = c["nc"]
    psel, pat, s32, s2 = c["consts"]
    bf = ml_dtypes.bfloat16

    # xt[p, ci, b] = x[b, 128ci + p]
    xt = np.ascontiguousarray(
        x.T.reshape(NCI, 128, B).transpose(1, 0, 2)).astype(bf)
    in_maps = []
    for cr in range(NCORES):
        t_c = T[:, OC * cr:OC * (cr + 1), :].reshape(IN, F)
        # tc[p, h, ci, fh] = T_c[128ci + p, 512h + fh]
        tcr = np.ascontiguousarray(
            t_c.reshape(NCI, 128, 2, 512).transpose(1, 2, 0, 3)).astype(bf)
        in_maps.append({"xt": xt, "tc": tcr, "psel": psel, "pat": pat,
                       "s32": s32, "s2": s2})

    kw = {}
    if _trace:
        kw = dict(trace=True, tmpdir=_tmpdir)
    res = run_bass_kernel_spmd(nc, in_maps, list(range(NCORES)), **kw)

    cntA = c["cntA"]
    i_map, o_map = c["i_map"], c["o_map"]
    o_b = np.empty((B, OUT), np.float32)
    for cr in range(NCORES):
        r = res.results[cr]
        po = r["po"]                              # [128, 192]
        poA = po[0:64, 0:128] - cntA[None, :]     # [o, j] junk-corrected
        ob_c = poA.T.copy()                       # [j, o]
        ob_c += po[:, 128:192]                    # B j-sums, junk-free
        rows = r["rs"].reshape(128, 32) - (i_map + 1)
        np.add.at(ob_c, (i_map.ravel(), o_map.ravel()), rows.ravel())
        o_b[:, OC * cr:OC * (cr + 1)] = ob_c
    out = np.concatenate([x, o_b], axis=1)
    if _trace:
        return out, res
    return out


# revision 17
# speedup vs baseline: 1.0323x; 1.0323x over previous
"""MinibatchDiscrimination kernel for 8 Trainium2 NeuronCores.

reference:
    m = einsum('bi,iok->bok', x, T)          # B=128, IN=1024, OUT=512, K=16
    norm[i,j,o] = sum_k |m[j,o,k] - m[i,o,k]|
    o_b = sum_i exp(-norm) - 1               # [B, OUT]
    out = concat([x, o_b], axis=1)           # [128, 1536]

Sharding: each core owns OUT/8 = 64 output features (zero communication).

Strictly-upper pairs (i<j) are split by i into two pipelines so all three
engines (PE / ACT / DVE) carry comparable load:

Pipeline A (i in blocks {0,1,6,7} of 16 -> 4064 pairs), per f-tile t:
  pair-diff on PE ([f, pairs] layout) -> |.| on ACT -> k-reduce on PE
  (packed selector matmuls, 4-strip interleave) -> exp on ACT over the
  [i-packed, j] grid -> col sums on PE (s2 selector) + row sums on DVE.
  PSUM pnorm grids are memset once and reuse the same written-mask, so
  stale cells stay exactly 0 -> exp gives exactly 1.0 junk, removed
  host-side via known counts.

Pipeline B (i in 32..95 -> 4064 pairs + 32 zero-pad), per 128-pair chunk:
  pair-diff transposed on PE (lhsT=psel chunk -> [pairs, f] f32 PSUM)
  -> fused |.|+k-reduce on DVE (tensor_reduce apply_absolute_value)
  -> exp on ACT ([pairs, o]) -> j-sum on PE (|psel|^T selector) which
  yields exact per-j sums with no junk (pad columns have zero weights).

Host combines: o_b[j, o] = A-colsums + A-rowsums (reindexed) + B-jsums.
"""

import numpy as np
import ml_dtypes

import concourse.bass as bass
import concourse.tile as tile
from concourse import mybir
from concourse.bass_utils import run_bass_kernel_spmd

BF16 = mybir.dt.bfloat16
F32 = mybir.dt.float32
A = mybir.AluOpType
AF = mybir.ActivationFunctionType
AX = mybir.AxisListType

B = 128
IN = 1024
OUT = 512
K = 16
NCORES = 8
OC = OUT // NCORES       # 64
F = OC * K               # 1024
NT = F // 128            # 8 f-tiles
NCI = IN // 128          # 8 contraction chunks

A_IGS = (0, 1, 6, 7)                    # 16-i blocks handled by pipeline A
B_IGS = tuple(g for g in range(8) if g not in A_IGS)
A_SET = tuple(i for g in A_IGS for i in range(16 * g, 16 * g + 16))
B_SET = tuple(i for g in B_IGS for i in range(16 * g, 16 * g + 16))
NPA = sum(127 - i for i in A_SET)       # 4064
NPB_RAW = sum(127 - i for i in B_SET)   # 4064
NBCH = (NPB_RAW + 127) // 128           # 32 B chunks
NPB = 128 * NBCH                        # padded with zero-selector columns
NPTOT = NPA + NPB                       # 8160
SUPW = 508
NSUP = NPA // SUPW                      # 8 supers per tile (exact: 4064=8*508)
assert NSUP * SUPW == NPA

# pair base offsets within the A region (i ascending over A_SET)
_BASE_A = {}
_off = 0
for _i in sorted(A_SET):
    _BASE_A[_i] = _off
    _off += 127 - _i


def _split_excess_waits(nc, max_waits=1):
    """This walrus build rejects instructions carrying more than one sem
    wait; hoist extras onto preceding NoOps on the same engine."""
    for fn in nc.m.functions:
        for blk in fn.blocks:
            new_insts = []
            for inst in blk.instructions:
                si = inst.sync_info
                if si and si.on_wait and len(si.on_wait) > max_waits:
                    waits = list(si.on_wait)
                    extra, keep = waits[:-max_waits], waits[-max_waits:]
                    k = 0
                    while extra:
                        chunk, extra = extra[:max_waits], extra[max_waits:]
                        nop = mybir.InstNoOp(
                            name=f"{inst.name}-ws{k}", engine=inst.engine,
                            ins=[], outs=[],
                            sync_info=mybir.SyncInfo(on_wait=chunk, on_update=[]))
                        nc.register_instruction(nop)
                        new_insts.append(nop)
                        k += 1
                    inst.sync_info = mybir.SyncInfo(
                        on_wait=keep, on_update=list(si.on_update))
                new_insts.append(inst)
            blk.instructions[:] = new_insts


def _build_program():
    nc = bass.Bass()
    xt_d = nc.dram_tensor("xt", [128, NCI, B], BF16, kind="ExternalInput")
    tc_d = nc.dram_tensor("tc", [128, 2, NCI, 512], BF16, kind="ExternalInput")
    psel_d = nc.dram_tensor("psel", [B, NPTOT], BF16, kind="ExternalInput")
    pat_d = nc.dram_tensor("pat", [128, NBCH, 128], BF16, kind="ExternalInput")
    s32_d = nc.dram_tensor("s32", [128, 4, 32], BF16, kind="ExternalInput")
    s2_d = nc.dram_tensor("s2", [128, NT, OC], BF16, kind="ExternalInput")
    po_d = nc.dram_tensor("po", [128, 192], F32, kind="ExternalOutput")
    rs_d = nc.dram_tensor("rs", [128, 32], F32, kind="ExternalOutput")

    with tile.TileContext(nc) as tc:
        with (
            tc.tile_pool(name="cst", bufs=1) as cst,
            tc.tile_pool(name="work", bufs=3) as work,
            tc.tile_pool(name="ework", bufs=4) as ework,
            tc.tile_pool(name="normb", bufs=2) as normb,
            tc.tile_pool(name="ebuf", bufs=2) as ebuf,
            tc.tile_pool(name="pdiff", bufs=2, space="PSUM") as pdiff,
            tc.tile_pool(name="pnorm", bufs=2, space="PSUM") as pnorm,
            tc.tile_pool(name="pbd", bufs=2, space="PSUM") as pbd,
            tc.tile_pool(name="pobA", bufs=1, space="PSUM") as pobA,
            tc.tile_pool(name="pobB", bufs=1, space="PSUM") as pobB,
        ):
            # ---- inputs: few large DMAs, dependency-ordered, two queues ----
            xt_sb = cst.tile([128, NCI, B], BF16, tag="xt")
            tc_sb = cst.tile([128, 2, NCI, 512], BF16, tag="tc")
            psel_sb = cst.tile([128, NPTOT], BF16, tag="psel")
            pat_sb = cst.tile([128, NBCH, 128], BF16, tag="pat")
            s32_sb = cst.tile([128, 4, 32], BF16, tag="s32")
            s2_sb = cst.tile([128, NT, OC], BF16, tag="s2")
            nc.sync.dma_start(xt_sb[:], xt_d[:])
            nc.sync.dma_start(tc_sb[:, 0], tc_d[:, 0])
            nc.sync.dma_start(tc_sb[:, 1], tc_d[:, 1])
            nc.scalar.dma_start(psel_sb[:, 0:NPA], psel_d[:, 0:NPA])
            nc.scalar.dma_start(s32_sb[:], s32_d[:])
            nc.scalar.dma_start(s2_sb[:], s2_d[:])
            nc.scalar.dma_start(psel_sb[:, NPA:NPTOT], psel_d[:, NPA:NPTOT])
            nc.scalar.dma_start(pat_sb[:], pat_d[:])

            # ---- GEMM: m[b, f] = x @ T_c (PSUM shared with pipeline B) ----
            m_bf = cst.tile([128, F], BF16, tag="mbf")
            for half in range(2):
                ps = pbd.tile([128, 512], F32, tag="bpd")
                for ci in range(NCI):
                    nc.tensor.matmul(
                        ps[:], xt_sb[:, ci, :], tc_sb[:, half, ci, :],
                        start=(ci == 0), stop=(ci == NCI - 1))
                nc.scalar.activation(m_bf[:, 512 * half:512 * (half + 1)],
                                     ps[:], AF.Copy, scale=1.0)

            po_a = pobA.tile([64, 512], F32, tag="poa")
            po_b = pobB.tile([128, 512], F32, tag="pob")
            rs_all = cst.tile([128, 4, 8], F32, tag="rs")

            # ---------- pipeline A helpers ----------
            def make_pd_steps(t):
                absd = work.tile([128, NPA], BF16, tag="absd")

                def step(s):
                    pd = pdiff.tile([128, 512], F32, tag="pd")
                    lo = s * SUPW
                    nc.tensor.matmul(pd[:, 0:SUPW],
                                     m_bf[:, 128 * t:128 * (t + 1)],
                                     psel_sb[:, lo:lo + SUPW],
                                     start=True, stop=True)
                    nc.scalar.activation(absd[:, lo:lo + SUPW],
                                         pd[:, 0:SUPW], AF.Abs)

                return absd, [lambda s=s: step(s) for s in range(NSUP)]

            # ---------- pipeline B helpers ----------
            bstate = {}

            def bstep(c):
                if c >= NBCH:
                    return
                if c % 4 == 0:
                    bstate["nb"] = normb.tile([128, 4, OC], F32, tag="nb",
                                              name="nb")
                nb = bstate["nb"]
                for h in range(2):
                    pd = pbd.tile([128, 512], F32, tag="bpd")
                    nc.tensor.matmul(pd[:], psel_sb[:, NPA + 128 * c:
                                                     NPA + 128 * (c + 1)],
                                     m_bf[:, 512 * h:512 * (h + 1)],
                                     start=True, stop=True)
                    nc.vector.tensor_reduce(
                        nb[:, c % 4, 32 * h:32 * (h + 1)],
                        pd[:].rearrange("p (o k) -> p o k", k=K),
                        op=A.add, axis=AX.X, apply_absolute_value=True)
                if c % 4 == 3:
                    eb = ebuf.tile([128, 4, OC], BF16, tag="eb", name="eb")
                    nc.scalar.activation(eb[:], nb[:], AF.Exp, scale=-1.0)
                    for k2 in range(4):
                        cc = (c // 4) * 4 + k2
                        nc.tensor.matmul(
                            po_b[:, 0:OC], pat_sb[:, cc, :], eb[:, k2, :],
                            start=(cc == 0), stop=(cc == NBCH - 1),
                            skip_group_check=True)

            # ---------- pipeline A k-reduce + exp + sums ----------
            def kred(t, absd, weave):
                weave = list(weave)
                stride = max(1, 64 // (len(weave) + 1)) if weave else 0
                mm_count = 0

                def tick():
                    nonlocal mm_count
                    mm_count += 1
                    if weave and stride and mm_count % stride == 0:
                        weave.pop(0)()

                pn = pnorm.tile([128, 512], F32, tag="pn")
                if t < 2:
                    # only the first use of each pnorm bank needs zeroing:
                    # later rounds rewrite the identical cell mask, so
                    # never-written cells keep this exact 0.0 forever
                    nc.vector.memset(pn[:], 0.0)
                first = True
                for gl in range(4):
                    ig = A_IGS[gl]
                    for idx in range(16):
                        q, a = idx % 4, idx // 4
                        i = 16 * ig + 4 * a + q
                        w = 127 - i
                        if w <= 0:
                            continue
                        bs = _BASE_A[i]
                        last = (gl == 3 and idx == 15)
                        nc.tensor.matmul(
                            pn[32 * q:32 * q + 32,
                               128 * gl + i + 1:128 * (gl + 1)],
                            s32_sb[:, a, :], absd[:, bs:bs + w],
                            start=first, stop=last,
                            tile_position=(0, 32 * q), skip_group_check=True)
                        first = False
                        tick()
                e = ework.tile([128, 512], BF16, tag="e")
                nc.scalar.activation(e[:], pn[:], AF.Exp, scale=-1.0)
                nc.vector.tensor_reduce(
                    rs_all[:, :, t],
                    e[:].rearrange("p (g j) -> p g j", g=4),
                    op=A.add, axis=AX.X)
                for gl in range(4):
                    nc.tensor.matmul(po_a[:, 0:128], s2_sb[:, t, :],
                                     e[:, 128 * gl:128 * (gl + 1)],
                                     start=(t == 0 and gl == 0),
                                     stop=(t == NT - 1 and gl == 3),
                                     skip_group_check=True)
                for stp in weave:
                    stp()

            # ---------- main schedule ----------
            absd0, steps0 = make_pd_steps(0)
            for s in steps0:
                s()
            cur_absd = absd0
            for t in range(NT):
                if t + 1 < NT:
                    nxt_absd, nxt_steps = make_pd_steps(t + 1)
                else:
                    nxt_absd, nxt_steps = None, []
                bsteps = [lambda c=c: bstep(c) for c in range(4 * t, 4 * t + 4)]
                # interleave pd (ACT abs) and B (DVE reduce) work so both
                # side engines stay fed during this tile's k-reduce
                weave = []
                for x_ in range(max(len(nxt_steps), len(bsteps))):
                    if x_ < len(nxt_steps):
                        weave.append(nxt_steps[x_])
                    if x_ < len(bsteps):
                        weave.append(bsteps[x_])
                kred(t, cur_absd, weave)
                cur_absd = nxt_absd

            po_sb = cst.tile([128, 192], F32, tag="posb")
            nc.vector.memset(po_sb[:], 0.0)
            nc.vector.tensor_copy(po_sb[0:64, 0:128], po_a[:, 0:128])
            nc.vector.tensor_copy(po_sb[:, 128:192], po_b[:, 0:OC])
            nc.sync.dma_start(po_d[:], po_sb[:])
            nc.sync.dma_start(rs_d[:], rs_all[:])

    _split_excess_waits(nc)
    return nc


def _host_consts():
    bf = ml_dtypes.bfloat16
    order = [(i, j) for i in sorted(A_SET) for j in range(i + 1, 128)]
    order += [(i, j) for i in sorted(B_SET) for j in range(i + 1, 128)]
    psel = np.zeros((B, NPTOT), np.float32)
    for col, (i, j) in enumerate(order):
        psel[i, col] = 1.0
        psel[j, col] = -1.0
    pat = np.zeros((128, NBCH, 128), np.float32)
    for c in range(NBCH):
        for r in range(128):
            gcol = NPA + 128 * c + r
            if gcol - NPA < NPB_RAW:
                i, j = order[gcol]
                pat[r, c, i] = 1.0
                pat[r, c, j] = 1.0
    s32 = np.zeros((128, 4, 32), np.float32)
    for a in range(4):
        for osub in range(8):
            s32[16 * osub:16 * (osub + 1), a, 8 * a + osub] = 1.0
    s2 = np.zeros((128, NT, OC), np.float32)
    for t in range(NT):
        for p in range(128):
            s2[p, t, 8 * t + (p % 8)] = 1.0
    return (psel.astype(bf), pat.astype(bf), s32.astype(bf), s2.astype(bf))


_CACHE = {}


def _get_cached():
    if "nc" not in _CACHE:
        _CACHE["nc"] = _build_program()
        _CACHE["consts"] = _host_consts()
        # A-side junk: unwritten (j <= i) cells read exp(0)=1.0
        jj = np.arange(B)
        a_arr = np.array(sorted(A_SET))
        _CACHE["cntA"] = (a_arr[None, :] >= jj[:, None]).sum(1).astype(np.float32)
        # rowsum reindex: rs[p, gl, t] belongs to i = 16*A_IGS[gl] + 4a + q
        # with p = 32q + 8a + osub, o = 8t + osub
        p_idx = np.arange(128)
        q, rem = p_idx // 32, p_idx % 32
        a_, osub = rem // 8, rem % 8
        cols = np.arange(32)
        gl, t_ = cols // 8, cols % 8
        igs = np.array(A_IGS)
        i_map = 16 * igs[gl][None, :] + 4 * a_[:, None] + q[:, None]
        o_map = 8 * t_[None, :] + osub[:, None]
        _CACHE["i_map"] = i_map
        _CACHE["o_map"] = o_map
    return _CACHE


def kernel(x: np.ndarray, T: np.ndarray, _trace=False, _tmpdir=None) -> np.ndarray:
    x = np.asarray(x, dtype=np.float32)
    T = np.asarray(T, dtype=np.float32)
    c = _get_cached()
    nc = c["nc"]
    psel, pat, s32, s2 = c["consts"]
    bf = ml_dtypes.bfloat16

    # xt[p, ci, b] = x[b, 128ci + p]
    xt = np.ascontiguousarray(
        x.T.reshape(NCI, 128, B).transpose(1, 0, 2)).astype(bf)
    in_maps = []
    for cr in range(NCORES):
        t_c = T[:, OC * cr:OC * (cr + 1), :].reshape(IN, F)
        # tc[p, h, ci, fh] = T_c[128ci + p, 512h + fh]
        tcr = np.ascontiguousarray(
            t_c.reshape(NCI, 128, 2, 512).transpose(1, 2, 0, 3)).astype(bf)
        in_maps.append({"xt": xt, "tc": tcr, "psel": psel, "pat": pat,
                       "s32": s32, "s2": s2})

    kw = {}
    if _trace:
        kw = dict(trace=True, tmpdir=_tmpdir)
    res = run_bass_kernel_spmd(nc, in_maps, list(range(NCORES)), **kw)

    cntA = c["cntA"]
    i_map, o_map = c["i_map"], c["o_map"]
    o_b = np.empty((B, OUT), np.float32)
    for cr in range(NCORES):
        r = res.results[cr]
        po = r["po"]                              # [128, 192]
        poA = po[0:64, 0:128] - cntA[None, :]     # [o, j] junk-corrected
        ob_c = poA.T.copy()                       # [j, o]
        ob_c += po[:, 128:192]                    # B j-sums, junk-free
        rows = r["rs"].reshape(128, 32) - (i_map + 1)
        np.add.at(ob_c, (i_map.ravel(), o_map.ravel()), rows.ravel())
        o_b[:, OC * cr:OC * (cr + 1)] = ob_c
    out = np.concatenate([x, o_b], axis=1)
    if _trace:
        return out, res
    return out
